# revision 1
# baseline (speedup 1.0000x reference)
"""Trainium2 Bass kernel for nn_ArithmeticNps (moe_routing).

Strategy
--------
Pure data parallel over 8 NeuronCores: each core processes B/8 = 2048
examples. Inside a core, the batch lives on the matmul *moving* (free)
dimension, features on partitions; the batch is processed in chunks of 512.

Routing-critical math (encoders -> selector1 -> selector2) runs in true
fp32 on the PE array (measured 1.6e-7 rel err; min top-2 argmax gap of the
problem is ~3e-5 so tf32 would flip routes). The heavy per-rule FFN and the
decoder run in float32r (tf32-class, 4x faster, ~1.6e-4 rel err - value
path only).

Argmax is never materialized as an integer: selector1's rule choice is
computed as a one-hot via (tree-max -> broadcast-by-matmul -> is_equal),
and rule selection in the FFN is done by adding -PEN*(1-onehot[r]) into
each rule's pre-activation via a rank-1 (K=1) accumulating matmul, so the
ReLU zeroes non-selected columns exactly and all 16 rule outputs can be
summed in one PSUM accumulation. Attention projections are pre-folded on
the host (weights-only algebra): att1 = (s1_q_w @ read1.T).T @ hidden and
att2_all = (s2_q_w[k] @ read2_table[r,n]).T @ hidden_k.
"""

import os
import sys

sys.path.insert(0, "/opt/trn_rl_repo")

import numpy as np

REPEAT = int(os.environ.get("NPS_REPEAT", "1"))

NCORES = 8
B_FULL = 16384
BC = B_FULL // NCORES  # per-core batch
CHUNK = 512
NCHUNK = BC // CHUNK
NR = 16  # rules
CV = 128  # concept vec
CR = 64  # rule emb dim
CM = 128  # rule hidden
PEN = 32768.0  # relu-mask penalty


def _host_prep(p):
    """Batch-independent weight algebra done on the host (fp32)."""
    f32 = np.float32
    w1e, b1e = p["enc_op_w1"], p["enc_op_b1"]
    w2e, b2e = p["enc_op_w2"], p["enc_op_b2"]
    w1o, b1o = p["enc_opr_w1"], p["enc_opr_b1"]
    w2o, b2o = p["enc_opr_w2"], p["enc_opr_b2"]

    # encoder layer-1 lhsT: X012 rows [op1; op2; ones], XO rows = onehot
    e1a = np.zeros((3, 128), f32)
    e1a[0, 0:64] = w1e[0]
    e1a[2, 0:64] = b1e
    e1a[1, 64:128] = w1e[0]
    e1a[2, 64:128] = w1e[1] + b1e
    e1b = (w1o + b1o[None, :]).astype(f32)  # (3, 64); b1o folded via sum(oh)=1

    read1 = np.einsum("nr,nrm->nm", p["rules_emb"], p["s1_k_w"]) + p["s1_k_b"]
    comb1 = (p["s1_q_w"] @ read1.T).astype(f32)  # (128, 16)
    att1b = (read1 @ p["s1_q_b"]).astype(f32)  # (16,)

    # read2_table[r, n] = rules_emb[r] @ s2_k_w[n] + s2_k_b[n]   (16, 2, 16)
    r2t = np.einsum("rc,ncm->rnm", p["rules_emb"], p["s2_k_w"]) + p["s2_k_b"]
    big2 = np.zeros((2, 128, 64), f32)
    att2b = np.zeros((64,), f32)
    for r in range(NR):
        for n in range(2):
            for k in range(2):
                j = 4 * r + 2 * n + k
                big2[k][:, j] = p["s2_q_w"][k] @ r2t[r, n]
                att2b[j] = r2t[r, n] @ p["s2_q_b"][k]

    rep4 = np.zeros((16, 64), f32)
    for r in range(NR):
        for i in range(4):
            rep4[r, 4 * r + i] = 1.0
    # fold to signed differences: col0 = s01-s00 (at out partition 0),
    # col32 = s11-s10 (at out partition 32)
    fold42 = np.zeros((64, 33), f32)
    for r in range(NR):
        fold42[4 * r + 1, 0] = 1.0
        fold42[4 * r + 0, 0] = -1.0
        fold42[4 * r + 3, 32] = 1.0
        fold42[4 * r + 2, 32] = -1.0

    # SBUF layout: (128 partitions, n_blocks*128 free); block i is lhsT_i
    rw1 = (p["rule_W1"].reshape(NR, 2, 128, CM).reshape(NR * 2, 128, CM)
           .transpose(1, 0, 2).reshape(128, NR * 2 * CM)).astype(f32)
    rw1 = np.ascontiguousarray(rw1)
    rw2 = np.ascontiguousarray(
        p["rule_W2"].transpose(1, 0, 2).reshape(128, NR * CV)).astype(f32)

    use_rb1 = bool(np.any(p["rule_b1"]))
    pT = np.zeros((17 if use_rb1 else 16, NR * 128), f32)
    for r in range(NR):
        pT[r, r * 128:(r + 1) * 128] = PEN
        if use_rb1:
            pT[16, r * 128:(r + 1) * 128] = p["rule_b1"][r]

    consts = {
        "penT": pT,
        "e1a": e1a,
        "e1b": e1b,
        "w2e2": np.vstack([w2e, w2e]).astype(f32),
        "w2o": w2o.astype(f32),
        "comb1": comb1,
        "att1b": att1b.reshape(16, 1),
        "big2a": big2[0],
        "big2b": big2[1],
        "att2b": att2b.reshape(64, 1),
        "rep4": rep4,
        "fold42": fold42,
        "ones_row": np.ones((1, 128), f32),
        "ones_c": np.ones((1, CHUNK), f32),
        "iota3": np.arange(3, dtype=f32).reshape(3, 1),
        "rw1": rw1,
        "rw2": rw2,
        "rb1": p["rule_b1"].astype(f32),
        "rb2": p["rule_b2"].astype(f32),
        "dec1": p["dec_w1"].astype(f32),
        "dec1b": p["dec_b1"].astype(f32).reshape(64, 1),
        "dec2": p["dec_w2"].astype(f32),
        "b2e": b2e.astype(f32).reshape(128, 1),
        "b2o": b2o.astype(f32).reshape(128, 1),
        "decb2": float(p["dec_b2"].reshape(-1)[0]),
    }
    return consts


def _build(consts):
    import concourse.bacc as bacc
    import concourse.tile as tile
    from concourse import bass_isa, mybir

    f32 = mybir.dt.float32
    f32r = mybir.dt.float32r
    AF = mybir.ActivationFunctionType
    ALU = mybir.AluOpType

    nc = bacc.Bacc("TRN2", target_bir_lowering=False, debug=False)

    # ---- DRAM I/O -------------------------------------------------------
    dj = {}

    def din(name, shape, dt=f32):
        dj[name] = nc.dram_tensor(name, list(shape), dt, kind="ExternalInput").ap()
        return dj[name]

    op1 = din("op1", (1, BC))
    op2 = din("op2", (1, BC))
    opr = din("opr", (1, BC))
    for nm in ("e1a", "e1b", "w2e2", "w2o", "comb1", "att1b", "big2a", "big2b",
               "att2b", "rep4", "fold42", "ones_row", "ones_c", "iota3",
               "dec1b", "b2e", "b2o"):
        din(nm, consts[nm].shape)
    for nm in ("penT", "rw1", "rw2", "dec1", "dec2"):
        din(nm, consts[nm].shape, f32r)
    out_d = nc.dram_tensor("out", [1, BC], f32, kind="ExternalOutput").ap()

    use_rb1 = bool(np.any(consts["rb1"]))
    use_rb2 = bool(np.any(consts["rb2"]))
    use_b2e = bool(np.any(consts["b2e"]))
    use_b2o = bool(np.any(consts["b2o"]))
    use_d1b = bool(np.any(consts["dec1b"]))
    if use_rb2:
        din("rb2t", (16, 128))  # transposed lhsT, built host-side

    C = CHUNK
    with tile.TileContext(nc) as tc:
        with tc.tile_pool(name="wsb", bufs=1) as wsb, \
             tc.tile_pool(name="xsb", bufs=2) as xsb, \
             tc.tile_pool(name="hsb", bufs=3) as hsb, \
             tc.tile_pool(name="asb", bufs=2) as asb, \
             tc.tile_pool(name="vsb", bufs=2) as vsb, \
             tc.tile_pool(name="msb", bufs=3) as msb, \
             tc.tile_pool(name="osb", bufs=2) as osb, \
             tc.tile_pool(name="penc", bufs=2, space="PSUM") as penc, \
             tc.tile_pool(name="prt", bufs=2, space="PSUM") as prt, \
             tc.tile_pool(name="ppre", bufs=2, space="PSUM") as ppre, \
             tc.tile_pool(name="psacc", bufs=2, space="PSUM") as psacc:

            # ---- load all weights/constants once ------------------------
            W = {}
            for nm, dt in [("e1a", f32), ("e1b", f32), ("w2e2", f32),
                           ("w2o", f32), ("comb1", f32), ("att1b", f32),
                           ("big2a", f32), ("big2b", f32), ("att2b", f32),
                           ("rep4", f32), ("fold42", f32), ("ones_row", f32),
                           ("iota3", f32), ("dec1b", f32), ("b2e", f32),
                           ("b2o", f32), ("penT", f32r), ("dec1", f32r),
                           ("dec2", f32r), ("rw1", f32r), ("rw2", f32r)]:
                sh = list(consts[nm].shape)
                t = wsb.tile(sh, dt, tag=f"w_{nm}")
                if nm == "rw1":
                    for i in range(4):
                        blk = slice(i * 1024, (i + 1) * 1024)
                        nc.gpsimd.dma_start(t[:, blk], dj[nm][:, blk])
                elif nm == "rw2":
                    for i in range(2):
                        blk = slice(i * 1024, (i + 1) * 1024)
                        nc.gpsimd.dma_start(t[:, blk], dj[nm][:, blk])
                elif nm in ("penT", "dec1", "dec2"):
                    nc.gpsimd.dma_start(t[:], dj[nm][:])
                else:
                    nc.sync.dma_start(t[:], dj[nm][:])
                W[nm] = t
            if use_rb2:
                t = wsb.tile([16, 128], f32, tag="w_rb2t")
                nc.sync.dma_start(t[:], dj["rb2t"][:])
                W["rb2t"] = t

            negone = wsb.tile([16, 1], f32, tag="negone")
            nc.vector.memset(negone[:], -1.0)

            use_a1b = bool(np.any(consts["att1b"]))

            for rep in range(REPEAT):
              for c in range(NCHUNK):
                cs = slice(c * C, (c + 1) * C)
                # ---- build X012 = [op1; op2; 1], XO = onehot(opr) -------
                X = xsb.tile([3, C], f32, tag="X")
                nc.sync.dma_start(X[0:1, :], op1[:, cs])
                nc.sync.dma_start(X[1:2, :], op2[:, cs])
                nc.sync.dma_start(X[2:3, :], dj["ones_c"][:])
                XO = xsb.tile([3, C], f32, tag="XO")
                for j in range(3):
                    nc.sync.dma_start(XO[j:j + 1, :], opr[:, cs])
                nc.vector.tensor_scalar(XO[:], XO[:], W["iota3"][:],
                                        None, op0=ALU.is_equal)

                # ---- encoders -------------------------------------------
                pre_a = penc.tile([128, C], f32, tag="enc")
                nc.tensor.matmul(pre_a[:], W["e1a"][:], X[:], start=True, stop=True)
                pre_b = penc.tile([64, C], f32, tag="enc")
                nc.tensor.matmul(pre_b[:], W["e1b"][:], XO[:], start=True, stop=True)
                r12 = xsb.tile([128, C], f32, tag="r12")
                nc.scalar.activation(r12[:], pre_a[:], AF.Relu)
                rO = xsb.tile([64, C], f32, tag="rO")
                nc.scalar.activation(rO[:], pre_b[:], AF.Relu)

                hid = hsb.tile([128, 3 * C], f32, tag="hid")
                for k, (lh, rh) in enumerate([
                        (W["w2e2"][0:64, :], r12[0:64, :]),
                        (W["w2e2"][64:128, :], r12[64:128, :]),
                        (W["w2o"][:], rO[:])]):
                    hps = penc.tile([128, C], f32, tag="enc")
                    nc.tensor.matmul(hps[:], lh, rh, start=True, stop=True)
                    bias = W["b2e"][:] if k < 2 else W["b2o"][:]
                    useb = use_b2e if k < 2 else use_b2o
                    if useb:
                        nc.scalar.activation(hid[:, k * C:(k + 1) * C], hps[:],
                                             AF.Identity, bias=bias)
                    else:
                        nc.vector.tensor_copy(hid[:, k * C:(k + 1) * C], hps[:])

                # ---- selector 1: rule one-hot ---------------------------
                attp = prt.tile([80, C], f32, tag="rt")
                for k in range(3):
                    nc.tensor.matmul(attp[32 * k:32 * k + 16, :], W["comb1"][:],
                                     hid[:, k * C:(k + 1) * C],
                                     start=True, stop=True,
                                     tile_position=(0, 32 * k))
                s0 = asb.tile([16, C], f32, tag="s0")
                nc.vector.tensor_copy(s0[:], attp[0:16, :])
                mA = asb.tile([16, C], f32, tag="mA")
                nc.vector.tensor_tensor(mA[:], s0[:], attp[32:48, :], op=ALU.max)
                m3 = asb.tile([16, C], f32, tag="m3")
                nc.vector.tensor_tensor(m3[:], mA[:], attp[64:80, :], op=ALU.max)
                if use_a1b:
                    nc.vector.tensor_scalar(m3[:], m3[:], W["att1b"][:], None,
                                            op0=ALU.add)
                mx16 = asb.tile([16, C], f32, tag="mx16")
                nc.gpsimd.partition_all_reduce(mx16[:], m3[:], channels=16,
                                               reduce_op=bass_isa.ReduceOp.max)
                oh16 = asb.tile([16, C], f32, tag="oh16")
                nc.vector.tensor_tensor(oh16[:], m3[:], mx16[:], op=ALU.is_equal)
                # ohm1 = onehot - 1 in f32r for the penalty matmul
                ohm1 = asb.tile([17 if use_rb1 else 16, C], f32r, tag="ohm1")
                nc.scalar.activation(ohm1[0:16, :], oh16[:], AF.Identity,
                                     bias=negone[:])
                if use_rb1:
                    nc.sync.dma_start(ohm1[16:17, :], dj["ones_c"][:])

                # ---- selector 2: var one-hots ---------------------------
                o64ps = prt.tile([64, C], f32, tag="rt")
                nc.tensor.matmul(o64ps[:], W["rep4"][:], oh16[:],
                                 start=True, stop=True)
                o64sb = asb.tile([64, C], f32, tag="o64sb")
                nc.scalar.copy(o64sb[:], o64ps[:])  # ACT (DVE busy with masks)
                att2ps = prt.tile([64, C], f32, tag="rt")
                nc.tensor.matmul(att2ps[:], W["big2a"][:], hid[:, 0:C],
                                 start=True, stop=False)
                nc.tensor.matmul(att2ps[:], W["big2b"][:], hid[:, C:2 * C],
                                 start=False, stop=True)
                mk = asb.tile([64, C], f32, tag="mk")
                nc.vector.scalar_tensor_tensor(mk[:], att2ps[:], W["att2b"][:],
                                               o64sb[:], op0=ALU.add,
                                               op1=ALU.mult)
                selps = prt.tile([33, C], f32, tag="rt")
                nc.tensor.matmul(selps[:], W["fold42"][:], mk[:],
                                 start=True, stop=True)
                selsb = asb.tile([33, C], f32, tag="selsb")
                nc.scalar.copy(selsb[:], selps[:])  # ACT
                selp = asb.tile([1, C], f32, tag="selp")
                nc.vector.tensor_scalar(selp[:], selsb[0:1, :], 0.0, None,
                                        op0=ALU.is_gt)
                selc = asb.tile([1, C], f32, tag="selc")
                nc.vector.tensor_scalar(selc[:], selsb[32:33, :], 0.0, None,
                                        op0=ALU.is_gt)

                sp128 = prt.tile([128, C], f32, tag="rt")
                nc.tensor.matmul(sp128[:], W["ones_row"][:], selp[:],
                                 start=True, stop=True)
                sc128 = prt.tile([128, C], f32, tag="rt")
                nc.tensor.matmul(sc128[:], W["ones_row"][:], selc[:],
                                 start=True, stop=True)
                d10 = hsb.tile([128, C], f32, tag="d10")
                nc.vector.tensor_tensor(d10[:], hid[:, C:2 * C], hid[:, 0:C],
                                        op=ALU.subtract)
                tp = hsb.tile([128, C], f32, tag="tp")
                nc.vector.tensor_tensor(tp[:], d10[:], sp128[:], op=ALU.mult)
                varp = vsb.tile([128, C], f32r, tag="varp")
                nc.vector.tensor_tensor(varp[:], hid[:, 0:C], tp[:], op=ALU.add)
                tq = hsb.tile([128, C], f32, tag="tq")
                nc.vector.tensor_tensor(tq[:], d10[:], sc128[:], op=ALU.mult)
                varc = vsb.tile([128, C], f32r, tag="varc")
                nc.vector.tensor_tensor(varc[:], hid[:, 0:C], tq[:], op=ALU.add)

                # ---- rule FFN (dense over rules, relu-penalty masked) ---
                out_ps = psacc.tile([128, C], f32, tag="acc")
                for r in range(NR):
                    pre = ppre.tile([128, C], f32, tag="pre")
                    nc.tensor.matmul(pre[:],
                                     W["rw1"][:, (2 * r) * CM:(2 * r + 1) * CM],
                                     varp[:], start=True, stop=False)
                    nc.tensor.matmul(pre[:],
                                     W["rw1"][:, (2 * r + 1) * CM:(2 * r + 2) * CM],
                                     varc[:], start=False, stop=False)
                    nc.tensor.matmul(pre[:],
                                     W["penT"][:, r * 128:(r + 1) * 128],
                                     ohm1[:], start=False, stop=True)
                    hm = msb.tile([128, C], f32r, tag="hm")
                    if r % 2 == 0:
                        nc.scalar.activation(hm[:], pre[:], AF.Relu)
                    else:
                        nc.vector.tensor_scalar(hm[:], pre[:], 0.0, None,
                                                op0=ALU.max)
                    nc.tensor.matmul(out_ps[:],
                                     W["rw2"][:, r * CV:(r + 1) * CV], hm[:],
                                     start=(r == 0),
                                     stop=(r == NR - 1 and not use_rb2))
                if use_rb2:
                    nc.tensor.matmul(out_ps[:], W["rb2t"][:], oh16[:],
                                     start=False, stop=True)

                # ---- decoder --------------------------------------------
                outsb = vsb.tile([128, C], f32r, tag="outsb")
                nc.scalar.copy(outsb[:], out_ps[:])
                d1ps = ppre.tile([64, C], f32, tag="pre")
                nc.tensor.matmul(d1ps[:], W["dec1"][:], outsb[:],
                                 start=True, stop=True)
                d1sb = vsb.tile([64, C], f32r, tag="d1sb")
                if use_d1b:
                    nc.scalar.activation(d1sb[:], d1ps[:], AF.Relu,
                                         bias=W["dec1b"][:])
                else:
                    nc.scalar.activation(d1sb[:], d1ps[:], AF.Relu)
                x3ps = ppre.tile([1, C], f32, tag="pre")
                nc.tensor.matmul(x3ps[:], W["dec2"][:], d1sb[:],
                                 start=True, stop=True)
                x3sb = osb.tile([1, C], f32, tag="x3")
                if consts["decb2"] != 0.0:
                    nc.scalar.activation(x3sb[:], x3ps[:], AF.Identity,
                                         bias=consts["decb2"])
                else:
                    nc.scalar.copy(x3sb[:], x3ps[:])
                nc.sync.dma_start(out_d[:, cs], x3sb[:])

    nc.compile()
    return nc


CONST_NAMES = ("e1a", "e1b", "w2e2", "w2o", "comb1", "att1b", "big2a",
               "big2b", "att2b", "rep4", "fold42", "ones_row", "ones_c",
               "iota3", "dec1b", "b2e", "b2o", "penT", "rw1", "rw2", "dec1",
               "dec2")


def _make_in_maps(consts, p):
    cmaps = {k: np.ascontiguousarray(consts[k]) for k in CONST_NAMES}
    if np.any(consts["rb2"]):
        cmaps["rb2t"] = np.ascontiguousarray(consts["rb2"])  # lhsT (16,128)
    op1 = p["operand1"].astype(np.float32).reshape(NCORES, 1, BC)
    op2 = p["operand2"].astype(np.float32).reshape(NCORES, 1, BC)
    opr = p["operator"].astype(np.float32).reshape(NCORES, 1, BC)
    in_maps = []
    for cidx in range(NCORES):
        m = dict(cmaps)
        m["op1"] = op1[cidx]
        m["op2"] = op2[cidx]
        m["opr"] = opr[cidx]
        in_maps.append(m)
    return in_maps


def kernel(**inputs):
    from concourse.bass_utils import run_bass_kernel_spmd

    p = {k: np.asarray(v) for k, v in inputs.items()}
    consts = _host_prep(p)
    nc = _build(consts)
    in_maps = _make_in_maps(consts, p)

    res = run_bass_kernel_spmd(nc, in_maps, core_ids=list(range(NCORES)))
    out = np.concatenate([res.results[i]["out"].reshape(-1)
                          for i in range(NCORES)])
    return out.astype(np.float32)


if __name__ == "__main__":
    sys.path.insert(0, "/root/problem")
    import reference as R

    inp = {k: np.asarray(v) for k, v in R.setup_inputs().items()}
    got = kernel(**inp)
    print("kernel output:", got.shape, got.dtype, got[:5])

